# revision 28
# baseline (speedup 1.0000x reference)
"""NodeDropout kernel for 8 trn2 NeuronCores — v3 "scatter-route".

out[e] = values[e] * keep[src[e]] * keep[dst[e]],  keep = ~nodes_flag (1M bools).

ap_gather costs ~27ns per stream index on this silicon (Q7 RD_CMD latency,
ReadOverlap=0), so per-edge gathers are hopeless (~17ms). Instead the table
is routed TO the edges with gpsimd.local_scatter, whose SBUF traffic is
fully sequential (~3.2us per instruction):

- keep bit-packed into 62500 uint16 half-words, sliced across partitions:
  T[p, x] = table16[128*x + p]  ([128, 490], ~1KB/partition, loaded once).
- A lookup (edge endpoint) with half-word index wh lives at partition
  wh % 128, slice index x = wh // 128, bit position id & 15.
- Host schedules each lookup to (batch b, slot s): the j-th user of a given
  (p, wh) gets b = j % NB, tile m = j // NB (m < K=2 guaranteed since no
  half-word has more than K*NB users whp). Slot s = running index within
  (p, b); capacity S_CAP with negligible overflow probability (asserted).
- Device, per batch: K local_scatters deliver T[p, x] into the slots that
  need them (idx tiles, -1 = unused); OR-merge; >> bp; & 1; * value.
- Two passes over the same NEFF: pass A computes v*keep[src] in src-slot
  layout; the host re-permutes that into dst-slot layout; pass B multiplies
  by keep[dst]. Host un-permutes the final slot grid to edge order.
"""
import numpy as np
import ml_dtypes
from contextlib import ExitStack

from concourse import bacc, mybir
from concourse import tile
from concourse.bass_utils import run_bass_kernel_spmd

P = 128
N_CORES = 8
NHALF = 62500                 # uint16 half-words = 1M bits
SLICE = 489                   # max halfword slice index is 488, zero-padded
K = 2                         # scatter tiles per batch (max users per (wh, b))
S_CAP = 560                   # slots per partition per batch (max 555 on these inputs)
NB = 37                       # batches: K*NB=74 >= max half-word popularity (checked by asserts)

_NC_CACHE = {}


def _build(nb):
    nc = bacc.Bacc()
    u16 = mybir.dt.uint16
    i16 = mybir.dt.int16
    f32 = mybir.dt.float32

    shr = mybir.AluOpType.logical_shift_right
    band = mybir.AluOpType.bitwise_and
    bor = mybir.AluOpType.bitwise_or
    mult = mybir.AluOpType.mult

    tab = nc.declare_dram_parameter("tab", [P, K * SLICE], u16, isOutput=False)
    idxs = nc.declare_dram_parameter("idxs", [nb, P, K * SLICE], i16, isOutput=False)
    bps = nc.declare_dram_parameter("bps", [nb, P, S_CAP], u16, isOutput=False)
    bf16 = mybir.dt.bfloat16
    va = nc.declare_dram_parameter("va", [nb, P, S_CAP], bf16, isOutput=False)
    out = nc.declare_dram_parameter("out", [nb, P, S_CAP], bf16, isOutput=True)

    with ExitStack() as ctx:
        tc = ctx.enter_context(tile.TileContext(nc))
        tp = ctx.enter_context(tc.tile_pool(name="t", bufs=1))
        sm = ctx.enter_context(tc.tile_pool(name="sm", bufs=5))

        tab_t = tp.tile([P, K * SLICE], u16)
        nc.sync.dma_start(tab_t[:], tab[:])

        for b in range(nb):
            ix_t = sm.tile([P, K * SLICE], i16, tag="ix")
            nc.sync.dma_start(ix_t[:], idxs[b])
            bp_t = sm.tile([P, S_CAP], u16, tag="bp")
            nc.scalar.dma_start(bp_t[:], bps[b])
            v_t = sm.tile([P, S_CAP], mybir.dt.bfloat16, tag="v")
            nc.scalar.dma_start(v_t[:], va[b])

            w0 = sm.tile([P, S_CAP], u16, tag="w0")
            nc.gpsimd.local_scatter(w0[:], tab_t[:], ix_t[:],
                                    channels=P, num_elems=S_CAP,
                                    num_idxs=K * SLICE)

            # bit = (w >> bp) & 1 ; out = bit * v
            nc.vector.tensor_tensor(w0[:], w0[:], bp_t[:], op=shr)
            nc.vector.tensor_scalar(w0[:], w0[:], 1, None, op0=band)
            o_t = sm.tile([P, S_CAP], mybir.dt.bfloat16, tag="o")
            nc.vector.tensor_tensor(o_t[:], w0[:], v_t[:], op=mult)
            nc.sync.dma_start(out[b], o_t[:])
    nc.finalize()
    return nc


def _schedule(ids):
    """Schedule one pass's lookups (node ids, [E]) to (batch, tile m, slot).

    Returns (flat_slot[E] into the [NB, P, S_CAP] grid, idx tiles
    [NB, P, K, SLICE] int16, bp tiles [NB, P, S_CAP] uint16).
    """
    E = ids.shape[0]
    wh = (ids >> 4).astype(np.int64)      # half-word index < 62500
    bp = (ids & 15).astype(np.uint16)
    p = wh % P
    x = wh // P                           # < SLICE

    order = np.argsort(wh, kind="stable")
    sw = wh[order]
    # rank j within each wh group
    grp_start = np.r_[0, np.flatnonzero(np.diff(sw)) + 1]
    gidx = np.repeat(np.arange(grp_start.size), np.diff(np.r_[grp_start, E]))
    j = np.arange(E) - grp_start[gidx]
    # per-word batch offset de-biases the round-robin (otherwise every word
    # with > NB users puts its extras in the low batches)
    off = (sw * 40503) % NB
    b = ((j + off) % NB).astype(np.int64)
    m = j // NB
    assert m.max() < K, f"half-word with more than {K * NB} users"

    p_s = p[order]
    # slot within (p, b)
    key = p_s * NB + b
    order2 = np.argsort(key, kind="stable")
    k2 = key[order2]
    g2_start = np.r_[0, np.flatnonzero(np.diff(k2)) + 1]
    g2idx = np.repeat(np.arange(g2_start.size), np.diff(np.r_[g2_start, E]))
    s2 = np.arange(E) - g2_start[g2idx]
    assert s2.max() < S_CAP, f"slot overflow {s2.max()}"
    s = np.empty(E, np.int64)
    s[order2] = s2

    # map back to original edge order
    e_of = order                          # sorted position -> edge
    flat_slot = np.empty(E, np.int64)
    flat_slot[e_of] = (b * P + p_s) * S_CAP + s

    idx_tiles = np.full((NB, P, K, SLICE), -1, np.int16)
    idx_tiles[b, p_s, m, x[e_of]] = s.astype(np.int16)

    bp_tiles = np.zeros((NB, P, S_CAP), np.uint16)
    bp_tiles.reshape(-1)[flat_slot[e_of]] = bp[e_of]
    return flat_slot, idx_tiles.reshape(NB, P, K * SLICE), bp_tiles


def prep(inputs):
    """Build (nc, per-core pass metadata) — shared with test.py."""
    edge_index = np.asarray(inputs["edge_index"])
    values = np.asarray(inputs["values"], dtype=np.float32)
    nodes_flag = np.asarray(inputs["nodes_flag"], dtype=bool)
    e_total = values.shape[0]
    assert e_total % N_CORES == 0
    e_per = e_total // N_CORES
    assert NB * P * S_CAP >= e_per

    if 0 not in _NC_CACHE:
        _NC_CACHE[0] = _build(NB)
    nc = _NC_CACHE[0]

    keep = ~nodes_flag
    keep_pad = np.zeros(NHALF * 16, dtype=bool)
    keep_pad[:keep.shape[0]] = keep
    t16 = np.packbits(keep_pad, bitorder="little").view(np.uint16)  # [62500]
    t16_pad = np.zeros(P * SLICE, np.uint16)
    t16_pad[:NHALF] = t16
    tab1 = t16_pad.reshape(SLICE, P).T                              # [128, 490]
    tab = np.ascontiguousarray(np.concatenate([tab1] * K, axis=1))  # [128, 980]

    ids = edge_index.astype(np.int64)
    cores = []
    for c in range(N_CORES):
        lo, hi = c * e_per, (c + 1) * e_per
        fsA, idxA, bpA = _schedule(ids[0, lo:hi])
        fsB, idxB, bpB = _schedule(ids[1, lo:hi])
        vaA = np.zeros((NB, P, S_CAP), ml_dtypes.bfloat16)
        vaA.reshape(-1)[fsA] = values[lo:hi]
        cores.append({"fsA": fsA, "fsB": fsB, "idxA": idxA, "idxB": idxB,
                      "bpA": bpA, "bpB": bpB, "vaA": vaA})
    return nc, {"tab": tab, "cores": cores, "e_per": e_per}


def _run_pass(nc, meta, which, va_list, trace=False):
    in_maps = []
    for c, m in enumerate(meta["cores"]):
        in_maps.append({
            "tab": meta["tab"],
            "idxs": m["idx" + which],
            "bps": m["bp" + which],
            "va": va_list[c],
        })
    return run_bass_kernel_spmd(nc, in_maps, list(range(N_CORES)), trace=trace)


def kernel(edge_index: np.ndarray, values: np.ndarray, nodes_flag: np.ndarray) -> np.ndarray:
    nc, meta = prep({"edge_index": edge_index, "values": values,
                     "nodes_flag": nodes_flag})
    cores = meta["cores"]

    resA = _run_pass(nc, meta, "A", [m["vaA"] for m in cores])

    # permute pass-A output (src-slot layout) into pass-B's dst-slot layout
    vaB = []
    for c, m in enumerate(cores):
        outA = resA.results[c]["out"].reshape(-1)
        v = np.zeros(NB * P * S_CAP, ml_dtypes.bfloat16)
        v[m["fsB"]] = outA[m["fsA"]]
        vaB.append(v.reshape(NB, P, S_CAP))
    resB = _run_pass(nc, meta, "B", vaB)

    outs = []
    for c, m in enumerate(cores):
        outB = resB.results[c]["out"].reshape(-1)
        outs.append(outB[m["fsB"]])
    return np.concatenate(outs).astype(np.float32)


if __name__ == "__main__":
    rng = np.random.default_rng(0)
    E = 20_000_000 // 8          # quick: one-core-sized problem per core
    E = 1_048_576 * 8
    N = 1_000_000
    ei = rng.integers(0, N, size=(2, E), dtype=np.int64)
    v = rng.random(E, dtype=np.float32)
    flag = rng.random(N) < 0.1
    got = kernel(ei, v, flag)
    keep = (~flag).astype(np.float32)
    exp = v * keep[ei[0]] * keep[ei[1]]
    err = np.max(np.abs(got - exp))
    print("max abs err:", err, "CORRECT:", np.allclose(got, exp))


# revision 29
# speedup vs baseline: 1.0200x; 1.0200x over previous
"""NodeDropout kernel for 8 trn2 NeuronCores — v3 "scatter-route".

out[e] = values[e] * keep[src[e]] * keep[dst[e]],  keep = ~nodes_flag (1M bools).

ap_gather costs ~27ns per stream index on this silicon (Q7 RD_CMD latency,
ReadOverlap=0), so per-edge gathers are hopeless (~17ms). Instead the table
is routed TO the edges with gpsimd.local_scatter, whose SBUF traffic is
fully sequential (~3.2us per instruction):

- keep bit-packed into 62500 uint16 half-words, sliced across partitions:
  T[p, x] = table16[128*x + p]  ([128, 490], ~1KB/partition, loaded once).
- A lookup (edge endpoint) with half-word index wh lives at partition
  wh % 128, slice index x = wh // 128, bit position id & 15.
- Host schedules each lookup to (batch b, slot s): the j-th user of a given
  (p, wh) gets b = j % NB, tile m = j // NB (m < K=2 guaranteed since no
  half-word has more than K*NB users whp). Slot s = running index within
  (p, b); capacity S_CAP with negligible overflow probability (asserted).
- Device, per batch: K local_scatters deliver T[p, x] into the slots that
  need them (idx tiles, -1 = unused); OR-merge; >> bp; & 1; * value.
- Two passes over the same NEFF: pass A computes v*keep[src] in src-slot
  layout; the host re-permutes that into dst-slot layout; pass B multiplies
  by keep[dst]. Host un-permutes the final slot grid to edge order.
"""
import numpy as np
import ml_dtypes
from contextlib import ExitStack

from concourse import bacc, mybir
from concourse import tile
from concourse.bass_utils import run_bass_kernel_spmd

P = 128
N_CORES = 8
NHALF = 62500                 # uint16 half-words = 1M bits
SLICE = 489                   # max halfword slice index is 488, zero-padded
K = 2                         # scatter tiles per batch (max users per (wh, b))
S_CAP = 580                   # slots per partition per batch (max 555 on these inputs)
NB = 37                       # batches: K*NB=74 >= max half-word popularity (checked by asserts)

_NC_CACHE = {}


def _build(nb):
    nc = bacc.Bacc()
    u16 = mybir.dt.uint16
    i16 = mybir.dt.int16
    f32 = mybir.dt.float32

    shr = mybir.AluOpType.logical_shift_right
    band = mybir.AluOpType.bitwise_and
    bor = mybir.AluOpType.bitwise_or
    mult = mybir.AluOpType.mult

    tab = nc.declare_dram_parameter("tab", [P, K * SLICE], u16, isOutput=False)
    idxs = nc.declare_dram_parameter("idxs", [nb, P, K * SLICE], i16, isOutput=False)
    bps = nc.declare_dram_parameter("bps", [nb, P, S_CAP], u16, isOutput=False)
    bf16 = mybir.dt.bfloat16
    va = nc.declare_dram_parameter("va", [nb, P, S_CAP], bf16, isOutput=False)
    out = nc.declare_dram_parameter("out", [nb, P, S_CAP], bf16, isOutput=True)

    with ExitStack() as ctx:
        tc = ctx.enter_context(tile.TileContext(nc))
        tp = ctx.enter_context(tc.tile_pool(name="t", bufs=1))
        sm = ctx.enter_context(tc.tile_pool(name="sm", bufs=5))

        tab_t = tp.tile([P, K * SLICE], u16)
        nc.sync.dma_start(tab_t[:], tab[:])

        for b in range(nb):
            ix_t = sm.tile([P, K * SLICE], i16, tag="ix")
            nc.sync.dma_start(ix_t[:], idxs[b])
            bp_t = sm.tile([P, S_CAP], u16, tag="bp")
            nc.scalar.dma_start(bp_t[:], bps[b])
            v_t = sm.tile([P, S_CAP], mybir.dt.bfloat16, tag="v")
            nc.scalar.dma_start(v_t[:], va[b])

            w0 = sm.tile([P, S_CAP], u16, tag="w0")
            nc.gpsimd.local_scatter(w0[:], tab_t[:], ix_t[:],
                                    channels=P, num_elems=S_CAP,
                                    num_idxs=K * SLICE)

            # bit = (w >> bp) & 1 ; out = bit * v
            nc.vector.tensor_tensor(w0[:], w0[:], bp_t[:], op=shr)
            nc.vector.tensor_scalar(w0[:], w0[:], 1, None, op0=band)
            o_t = sm.tile([P, S_CAP], mybir.dt.bfloat16, tag="o")
            nc.vector.tensor_tensor(o_t[:], w0[:], v_t[:], op=mult)
            nc.sync.dma_start(out[b], o_t[:])
    nc.finalize()
    return nc


def _schedule(ids):
    """Schedule one pass's lookups (node ids, [E]) to (batch, tile m, slot).

    Returns (flat_slot[E] into the [NB, P, S_CAP] grid, idx tiles
    [NB, P, K, SLICE] int16, bp tiles [NB, P, S_CAP] uint16).
    """
    E = ids.shape[0]
    wh = (ids >> 4).astype(np.int64)      # half-word index < 62500
    bp = (ids & 15).astype(np.uint16)
    p = wh % P
    x = wh // P                           # < SLICE

    order = np.argsort(wh, kind="stable")
    sw = wh[order]
    # rank j within each wh group
    grp_start = np.r_[0, np.flatnonzero(np.diff(sw)) + 1]
    gidx = np.repeat(np.arange(grp_start.size), np.diff(np.r_[grp_start, E]))
    j = np.arange(E) - grp_start[gidx]
    # per-word batch offset de-biases the round-robin (otherwise every word
    # with > NB users puts its extras in the low batches)
    off = (sw * 40503) % NB
    b = ((j + off) % NB).astype(np.int64)
    m = j // NB
    assert m.max() < K, f"half-word with more than {K * NB} users"

    p_s = p[order]
    # slot within (p, b)
    key = p_s * NB + b
    order2 = np.argsort(key, kind="stable")
    k2 = key[order2]
    g2_start = np.r_[0, np.flatnonzero(np.diff(k2)) + 1]
    g2idx = np.repeat(np.arange(g2_start.size), np.diff(np.r_[g2_start, E]))
    s2 = np.arange(E) - g2_start[g2idx]
    assert s2.max() < S_CAP, f"slot overflow {s2.max()}"
    s = np.empty(E, np.int64)
    s[order2] = s2

    # map back to original edge order
    e_of = order                          # sorted position -> edge
    flat_slot = np.empty(E, np.int64)
    flat_slot[e_of] = (b * P + p_s) * S_CAP + s

    idx_tiles = np.full((NB, P, K, SLICE), -1, np.int16)
    idx_tiles[b, p_s, m, x[e_of]] = s.astype(np.int16)

    bp_tiles = np.zeros((NB, P, S_CAP), np.uint16)
    bp_tiles.reshape(-1)[flat_slot[e_of]] = bp[e_of]
    return flat_slot, idx_tiles.reshape(NB, P, K * SLICE), bp_tiles


def prep(inputs):
    """Build (nc, per-core pass metadata) — shared with test.py."""
    edge_index = np.asarray(inputs["edge_index"])
    values = np.asarray(inputs["values"], dtype=np.float32)
    nodes_flag = np.asarray(inputs["nodes_flag"], dtype=bool)
    e_total = values.shape[0]
    assert e_total % N_CORES == 0
    e_per = e_total // N_CORES
    assert NB * P * S_CAP >= e_per

    if 0 not in _NC_CACHE:
        _NC_CACHE[0] = _build(NB)
    nc = _NC_CACHE[0]

    keep = ~nodes_flag
    keep_pad = np.zeros(NHALF * 16, dtype=bool)
    keep_pad[:keep.shape[0]] = keep
    t16 = np.packbits(keep_pad, bitorder="little").view(np.uint16)  # [62500]
    t16_pad = np.zeros(P * SLICE, np.uint16)
    t16_pad[:NHALF] = t16
    tab1 = t16_pad.reshape(SLICE, P).T                              # [128, 490]
    tab = np.ascontiguousarray(np.concatenate([tab1] * K, axis=1))  # [128, 980]

    ids = edge_index.astype(np.int64)
    cores = []
    for c in range(N_CORES):
        lo, hi = c * e_per, (c + 1) * e_per
        fsA, idxA, bpA = _schedule(ids[0, lo:hi])
        fsB, idxB, bpB = _schedule(ids[1, lo:hi])
        vaA = np.zeros((NB, P, S_CAP), ml_dtypes.bfloat16)
        vaA.reshape(-1)[fsA] = values[lo:hi]
        cores.append({"fsA": fsA, "fsB": fsB, "idxA": idxA, "idxB": idxB,
                      "bpA": bpA, "bpB": bpB, "vaA": vaA})
    return nc, {"tab": tab, "cores": cores, "e_per": e_per}


def _run_pass(nc, meta, which, va_list, trace=False):
    in_maps = []
    for c, m in enumerate(meta["cores"]):
        in_maps.append({
            "tab": meta["tab"],
            "idxs": m["idx" + which],
            "bps": m["bp" + which],
            "va": va_list[c],
        })
    return run_bass_kernel_spmd(nc, in_maps, list(range(N_CORES)), trace=trace)


def kernel(edge_index: np.ndarray, values: np.ndarray, nodes_flag: np.ndarray) -> np.ndarray:
    nc, meta = prep({"edge_index": edge_index, "values": values,
                     "nodes_flag": nodes_flag})
    cores = meta["cores"]

    resA = _run_pass(nc, meta, "A", [m["vaA"] for m in cores])

    # permute pass-A output (src-slot layout) into pass-B's dst-slot layout
    vaB = []
    for c, m in enumerate(cores):
        outA = resA.results[c]["out"].reshape(-1)
        v = np.zeros(NB * P * S_CAP, ml_dtypes.bfloat16)
        v[m["fsB"]] = outA[m["fsA"]]
        vaB.append(v.reshape(NB, P, S_CAP))
    resB = _run_pass(nc, meta, "B", vaB)

    outs = []
    for c, m in enumerate(cores):
        outB = resB.results[c]["out"].reshape(-1)
        outs.append(outB[m["fsB"]])
    return np.concatenate(outs).astype(np.float32)


if __name__ == "__main__":
    rng = np.random.default_rng(0)
    E = 20_000_000 // 8          # quick: one-core-sized problem per core
    E = 1_048_576 * 8
    N = 1_000_000
    ei = rng.integers(0, N, size=(2, E), dtype=np.int64)
    v = rng.random(E, dtype=np.float32)
    flag = rng.random(N) < 0.1
    got = kernel(ei, v, flag)
    keep = (~flag).astype(np.float32)
    exp = v * keep[ei[0]] * keep[ei[1]]
    err = np.max(np.abs(got - exp))
    print("max abs err:", err, "CORRECT:", np.allclose(got, exp))
